# revision 1
# baseline (speedup 1.0000x reference)
"""CrossAttention kernel for 8 Trainium2 NeuronCores (Bass/Tile).

Sharding: tensor-parallel over heads. Core i handles heads {2i, 2i+1} for
both batch elements. LayerNorm scale/bias and the q-scale are folded into
the projection weights on the host; the per-token LN affine (1/sigma, mu)
is applied on-device (sums via PE ones-matmuls, mu-correction as an extra
K=1 contraction row). Scores are computed transposed [key, q] so the
attention-weighted sum over keys maps onto the PE contraction axis; the
softmax denominator rides the AV matmul as a ones-column of V.
Host gather: sum the 8 partial [dout, tok] projections, transpose back.
"""

import os
import sys

for _p in ("/opt/trn_rl_repo", "/root/.axon_site/_ro/trn_rl_repo"):
    if os.path.isdir(_p) and _p not in sys.path:
        sys.path.insert(0, _p)

import numpy as np
import ml_dtypes

import concourse.bass as bass
import concourse.tile as tile
from concourse import bacc, mybir
from concourse.masks import make_identity

BF16 = ml_dtypes.bfloat16

HEADS = 16
N_CORES = 8
H_PER_CORE = HEADS // N_CORES  # 2
DH = 64
LN_EPS = 1e-5

B = 2
N_TOK = 2048
D = 1024

QT = 512            # query tile (free dim of scores matmuls)
KT = 128            # key tile (partition dim of scoresT)
TT = 512            # token tile for LN/projection phase
N_DT = D // 128     # 8 contraction tiles of 128 over d


def build_program(n_tok=N_TOK):
    """Build the single-core SPMD Bass program. Returns nc."""
    nc = bacc.Bacc("TRN2")
    f32 = mybir.dt.float32
    f32r = mybir.dt.float32r
    bf16 = mybir.dt.bfloat16

    n_tt = n_tok // TT          # token tiles per batch
    n_qt = n_tok // QT          # query tiles per batch
    n_kt = n_tok // KT          # key tiles per batch

    # ---- DRAM parameters (per-core shards, host-prepped) ----
    xT = nc.declare_dram_parameter("xT", [B, D, n_tok], bf16, isOutput=False)
    cT = nc.declare_dram_parameter("cT", [B, D, n_tok], bf16, isOutput=False)
    alibiT = nc.declare_dram_parameter(
        "alibiT", [H_PER_CORE, n_tok, n_tok], f32r, isOutput=False)
    identf = nc.declare_dram_parameter("identf", [128, 128], f32r, isOutput=False)
    wqT = nc.declare_dram_parameter("wqT", [D, 128], bf16, isOutput=False)
    wkT = nc.declare_dram_parameter("wkT", [D, 128], bf16, isOutput=False)
    wvT = nc.declare_dram_parameter("wvT", [D, 128], bf16, isOutput=False)
    # rows: -wbar_q, -wbar_k, -wbar_v   (lhsT for the K=1 mu-correction row)
    wbar = nc.declare_dram_parameter("wbar", [3, 128], bf16, isOutput=False)
    woT = nc.declare_dram_parameter("woT", [128, D], bf16, isOutput=False)
    # columns: q/k/v projection bias (ln_b folded through W), fp32
    pbias = nc.declare_dram_parameter("pbias", [128, 3], f32, isOutput=False)
    bo_r = nc.declare_dram_parameter("bo_r", [128, N_DT], f32, isOutput=False)

    outT = nc.declare_dram_parameter(
        "outT", [D, B * n_tok], f32, isOutput=True)

    xT_r = xT.rearrange("b (dt p) n -> b p dt n", p=128)
    cT_r = cT.rearrange("b (dt p) n -> b p dt n", p=128)
    woT_r = woT.rearrange("c (dt n) -> c dt n", n=128)

    with tile.TileContext(nc) as tc:
        with tc.tile_pool(name="const", bufs=1) as const_pool:
            ident = const_pool.tile([128, 128], bf16)
            make_identity(nc, ident)
            ident_f = const_pool.tile([128, 128], f32r)
            nc.sync.dma_start(out=ident_f, in_=identf[:, :])
            zeros128 = const_pool.tile([128, 1], f32)
            nc.vector.memset(zeros128, 0.0)
            eps4 = const_pool.tile([4, 1], f32)
            nc.vector.memset(eps4, LN_EPS)
            # stats lhsT: onehot[:, u, j] is all-ones iff j == u
            onehot = const_pool.tile([128, n_tt, 4], bf16)
            nc.vector.memset(onehot, 0.0)
            for u in range(n_tt):
                nc.vector.memset(onehot[:, u, u:u + 1], 1.0)

            wq_sb = const_pool.tile([128, N_DT, 128], bf16)
            wk_sb = const_pool.tile([128, N_DT, 128], bf16)
            wv_sb = const_pool.tile([128, N_DT, 128], bf16)
            nc.sync.dma_start(out=wq_sb, in_=wqT.rearrange("(dt p) c -> p dt c", p=128))
            nc.sync.dma_start(out=wk_sb, in_=wkT.rearrange("(dt p) c -> p dt c", p=128))
            nc.sync.dma_start(out=wv_sb, in_=wvT.rearrange("(dt p) c -> p dt c", p=128))
            wbar_sb = const_pool.tile([1, 3, 128], bf16)
            nc.sync.dma_start(out=wbar_sb, in_=wbar[None, :, :])
            wo_sb = const_pool.tile([128, N_DT, 128], bf16)
            nc.sync.dma_start(out=wo_sb, in_=woT_r)
            pbias_sb = const_pool.tile([128, 3], f32)
            nc.sync.dma_start(out=pbias_sb, in_=pbias[:, :])
            bo_sb = const_pool.tile([128, N_DT], f32)
            nc.sync.dma_start(out=bo_sb, in_=bo_r[:, :])

            # persistent activations (bf16): [c(128), b, tok]
            qT_sb = const_pool.tile([128, B, n_tok], f32r)
            kT_sb = const_pool.tile([128, B, n_tok], f32r)
            vT_sb = const_pool.tile([128, B, n_tok], bf16)
            # v natural (+ones col): [key(128), b*n_kt*h, 66]
            vaug_sb = const_pool.tile([128, B * n_kt * H_PER_CORE, 66], bf16)
            nc.vector.memset(vaug_sb[:, :, 64:65], 1.0)

            def vaug_idx(b, kt, h):
                return (b * n_kt + kt) * H_PER_CORE + h

            # ============ Phase A: LN stats + apply + QKV projections ========
            with tc.tile_pool(name="raw_p", bufs=n_tt + 3) as raw_p, \
                 tc.tile_pool(name="pha", bufs=2) as pha, \
                 tc.tile_pool(name="pha_ps", bufs=2, space="PSUM") as pha_ps, \
                 tc.tile_pool(name="stat_ps", bufs=2, space="PSUM") as stat_ps, \
                 tc.tile_pool(name="stat_sb", bufs=2) as stat_sb, \
                 tc.tile_pool(name="vt_ps", bufs=2, space="PSUM") as vt_ps:
                for src_i, src_r in ((0, xT_r), (1, cT_r)):
                    for b in range(B):
                        # --- stats accumulation across the token tiles ---
                        sx = stat_ps.tile([4, TT], f32, tag="sx")
                        sxx = stat_ps.tile([4, TT], f32, tag="sxx")
                        raws = []
                        for u in range(n_tt):
                            raw = raw_p.tile([128, N_DT, TT], bf16, tag="raw")
                            raws.append(raw)
                            nc.sync.dma_start(
                                out=raw, in_=src_r[b, :, :, u * TT:(u + 1) * TT])
                            sq = pha.tile([128, N_DT, TT], bf16, tag="sq")
                            nc.vector.tensor_mul(sq, raw, raw)
                            for dt in range(N_DT):
                                first = (u == 0 and dt == 0)
                                last = (u == n_tt - 1 and dt == N_DT - 1)
                                nc.tensor.matmul(
                                    sx, onehot[:, u, :], raw[:, dt, :],
                                    start=first, stop=last)
                                nc.tensor.matmul(
                                    sxx, onehot[:, u, :], sq[:, dt, :],
                                    start=first, stop=last)
                        # --- batched LN math on [n_tt, TT] rows ---
                        e = stat_sb.tile([4, TT], f32, tag="e")
                        nc.vector.tensor_scalar_mul(e, sx, 1.0 / D)
                        ee = stat_sb.tile([4, TT], f32, tag="ee")
                        nc.vector.tensor_mul(ee, e, e)
                        var = stat_sb.tile([4, TT], f32, tag="var")
                        # var*D = Sxx - D*ee
                        nc.vector.scalar_tensor_tensor(
                            out=var, in0=ee, scalar=float(-D), in1=sxx,
                            op0=mybir.AluOpType.mult, op1=mybir.AluOpType.add)
                        lnv = stat_sb.tile([4, TT], f32, tag="lnv")
                        nc.scalar.activation(
                            out=lnv, in_=var, func=mybir.ActivationFunctionType.Ln,
                            bias=eps4[:, 0:1], scale=1.0 / D)
                        invs = stat_sb.tile([4, TT], f32, tag="invs")
                        nc.scalar.activation(
                            out=invs, in_=lnv, func=mybir.ActivationFunctionType.Exp,
                            bias=zeros128[0:4, 0:1], scale=-0.5)
                        invs_bf = stat_sb.tile([4, TT], bf16, tag="invs_bf")
                        nc.vector.tensor_copy(invs_bf, invs)
                        m_bf = stat_sb.tile([4, TT], bf16, tag="m_bf")
                        nc.vector.tensor_mul(m_bf, e, invs)
                        # restage rows at partition 0 (matmul rhs and
                        # partition_broadcast both need base partition 0)
                        m_row = stat_sb.tile([1, n_tt, TT], bf16, tag="m_row")
                        inv_row = stat_sb.tile([1, n_tt, TT], bf16, tag="inv_row")
                        for u in range(n_tt):
                            nc.sync.dma_start(
                                out=m_row[:, u, :], in_=m_bf[u:u + 1, :])
                            nc.sync.dma_start(
                                out=inv_row[:, u, :], in_=invs_bf[u:u + 1, :])

                        # --- apply + projections per token tile ---
                        for u in range(n_tt):
                            isb = pha.tile([128, TT], bf16, tag="isb")
                            nc.gpsimd.partition_broadcast(
                                isb, inv_row[:, u, :])
                            y = pha.tile([128, N_DT, TT], bf16, tag="y")
                            for dt in range(N_DT):
                                nc.vector.tensor_mul(
                                    y[:, dt, :], raws[u][:, dt, :], isb)
                            if src_i == 0:
                                projs = ((0, wq_sb, qT_sb),)
                            else:
                                projs = ((1, wk_sb, kT_sb), (2, wv_sb, vT_sb))
                            for wi, w_sb, dst in projs:
                                ps = pha_ps.tile([128, TT], f32, tag="proj")
                                for dt in range(N_DT):
                                    nc.tensor.matmul(
                                        ps, w_sb[:, dt, :], y[:, dt, :],
                                        start=(dt == 0), stop=False)
                                nc.tensor.matmul(
                                    ps, wbar_sb[:, wi, :], m_row[:, u, :],
                                    start=False, stop=True)
                                nc.scalar.activation(
                                    out=dst[:, b, u * TT:(u + 1) * TT], in_=ps,
                                    func=mybir.ActivationFunctionType.Identity,
                                    bias=pbias_sb[:, wi:wi + 1], scale=1.0)
                        # --- v natural (transpose vT) once per ctx batch ---
                        if src_i == 1:
                            for kt in range(n_kt):
                                vt = vt_ps.tile([128, 128], bf16, tag="vt")
                                nc.tensor.transpose(
                                    vt, vT_sb[:, b, kt * KT:(kt + 1) * KT], ident)
                                for h in range(H_PER_CORE):
                                    nc.vector.tensor_copy(
                                        vaug_sb[:, vaug_idx(b, kt, h), 0:64],
                                        vt[:, h * 64:(h + 1) * 64])

            # ============ Phase B: attention + output projection =============
            with tc.tile_pool(name="phb", bufs=6) as phb, \
                 tc.tile_pool(name="phbm", bufs=3) as phbm, \
                 tc.tile_pool(name="phfo", bufs=2) as phfo, \
                 tc.tile_pool(name="ps512", bufs=4, space="PSUM") as ps512, \
                 tc.tile_pool(name="av_ps", bufs=1, space="PSUM") as av_ps, \
                 tc.tile_pool(name="phb2", bufs=3) as phb2:
                for qt in range(n_qt):
                    q_sl = slice(qt * QT, (qt + 1) * QT)
                    av = [[av_ps.tile([65, QT], f32, tag=f"av{b}{h}",
                                      name=f"av{b}{h}")
                           for h in range(H_PER_CORE)] for b in range(B)]
                    exs = {}
                    for kt in range(n_kt):
                        k_sl = slice(kt * KT, (kt + 1) * KT)
                        al = phb.tile([128, H_PER_CORE, QT], f32r, tag="al")
                        nc.sync.dma_start(
                            out=al, in_=alibiT[:, k_sl, q_sl].rearrange(
                                "h p n -> p h n"))
                        for b in range(B):
                            ex = phb.tile([128, H_PER_CORE, QT], bf16,
                                          tag=f"ex{b}", name=f"ex{b}")
                            # balance the alibi add between PE (identity
                            # matmul) and DVE (tensor_tensor); on odd
                            # key-tiles both heads take the DVE path and
                            # share a single exp over [128, 2, QT]
                            merged = (kt % 2 == 1)
                            scs2 = None
                            if merged:
                                scs2 = phbm.tile([128, H_PER_CORE, QT], f32,
                                                 tag="scs2")
                            for h in range(H_PER_CORE):
                                c_sl = slice(h * 64, (h + 1) * 64)
                                sc = ps512.tile([128, QT], f32, tag="ps512",
                                                name="sc")
                                on_dve = merged or h == 1
                                nc.tensor.matmul(
                                    sc, kT_sb[c_sl, b, k_sl],
                                    qT_sb[c_sl, b, q_sl],
                                    start=True, stop=on_dve,
                                    tile_position=(h * 64, 0))
                                if merged:
                                    nc.vector.tensor_add(
                                        scs2[:, h, :], sc,
                                        al[:, h, :].bitcast(f32))
                                    continue
                                if on_dve:
                                    scs = phb.tile([128, QT], f32, tag="scs")
                                    nc.vector.tensor_add(
                                        scs, sc, al[:, h, :].bitcast(f32))
                                    exp_in = scs
                                else:
                                    nc.tensor.matmul(
                                        sc, ident_f[:, :], al[:, h, :],
                                        start=False, stop=True)
                                    exp_in = sc
                                nc.scalar.activation(
                                    out=ex[:, h, :], in_=exp_in,
                                    func=mybir.ActivationFunctionType.Exp,
                                    bias=zeros128[:, 0:1], scale=1.0)
                            if merged:
                                nc.scalar.activation(
                                    out=ex, in_=scs2,
                                    func=mybir.ActivationFunctionType.Exp,
                                    bias=zeros128[:, 0:1], scale=1.0)
                            for h in range(H_PER_CORE):
                                nc.tensor.matmul(
                                    av[b][h],
                                    vaug_sb[:, vaug_idx(b, kt, h), 0:65],
                                    ex[:, h, :],
                                    start=(kt == 0), stop=(kt == n_kt - 1))
                    for b in range(B):
                        # normalize: rows 0:64 outT_h, row 64 the denominator
                        den = phb2.tile([1, 2, QT], f32, tag="den")
                        for h in range(H_PER_CORE):
                            nc.vector.tensor_copy(den[:, h, :],
                                                  av[b][h][64:65, :])
                        rden = phb2.tile([1, 2, QT], f32, tag="rden")
                        nc.vector.reciprocal_approx_fast(rden, den)
                        o_sb = phb2.tile([128, QT], bf16, tag="o_sb")
                        for h in range(H_PER_CORE):
                            rb = phb2.tile([64, QT], f32, tag=f"rb{h}")
                            nc.gpsimd.partition_broadcast(rb, rden[:, h, :])
                            nc.vector.tensor_mul(
                                o_sb[h * 64:(h + 1) * 64, :],
                                av[b][h][0:64, :], rb)
                        # output projection: [dout, q] partials
                        fo = phfo.tile([128, N_DT, QT], f32, tag="fo")
                        for dt in range(N_DT):
                            fp = ps512.tile([128, QT], f32, tag="ps512",
                                            name="fp")
                            nc.tensor.matmul(
                                fp, wo_sb[:, dt, :], o_sb, start=True,
                                stop=True)
                            nc.scalar.activation(
                                out=fo[:, dt, :], in_=fp,
                                func=mybir.ActivationFunctionType.Identity,
                                bias=bo_sb[:, dt:dt + 1], scale=1.0)
                        nc.sync.dma_start(
                            out=outT.rearrange("(dt p) n -> p dt n", p=128)[
                                :, :, b * n_tok + qt * QT:
                                b * n_tok + (qt + 1) * QT],
                            in_=fo)
    nc.compile()
    return nc


_NC_CACHE = {}


def _get_program(n_tok=N_TOK):
    if n_tok not in _NC_CACHE:
        _NC_CACHE[n_tok] = build_program(n_tok)
    return _NC_CACHE[n_tok]


def _prep_in_maps(x, context, alibi, Wq, Wk, Wv, Wo, bo, ln_w, ln_b):
    b, n, d = x.shape
    scale = (d // HEADS) ** -0.5

    x = np.asarray(x, dtype=np.float32)
    context = np.asarray(context, dtype=np.float32)
    alibi = np.asarray(alibi, dtype=np.float32)
    Wq, Wk, Wv, Wo = (np.asarray(w, dtype=np.float32) for w in (Wq, Wk, Wv, Wo))
    bo = np.asarray(bo, dtype=np.float32)
    ln_w = np.asarray(ln_w, dtype=np.float32)
    ln_b = np.asarray(ln_b, dtype=np.float32)

    xT = np.ascontiguousarray(x.transpose(0, 2, 1)).astype(BF16)
    cT = np.ascontiguousarray(context.transpose(0, 2, 1)).astype(BF16)

    in_maps = []
    for ci in range(N_CORES):
        h0 = ci * H_PER_CORE
        cs = slice(h0 * DH, (h0 + H_PER_CORE) * DH)  # this core's 128 channels
        alT = np.ascontiguousarray(
            alibi[0, h0:h0 + H_PER_CORE].transpose(0, 2, 1)).astype(np.float32)

        wq_s = (Wq[cs] * ln_w[None, :]) * scale          # [128, d]
        wk_s = Wk[cs] * ln_w[None, :]
        wv_s = Wv[cs] * ln_w[None, :]
        wbar = np.stack([
            -wq_s.sum(axis=1), -wk_s.sum(axis=1), -wv_s.sum(axis=1)])
        pb = np.stack([
            (Wq[cs] @ ln_b) * scale, Wk[cs] @ ln_b, Wv[cs] @ ln_b]).T  # [128,3]

        bo_core = bo if ci == 0 else np.zeros_like(bo)

        in_maps.append({
            "xT": xT,
            "cT": cT,
            "alibiT": alT,
            "wqT": np.ascontiguousarray(wq_s.T).astype(BF16),
            "wkT": np.ascontiguousarray(wk_s.T).astype(BF16),
            "wvT": np.ascontiguousarray(wv_s.T).astype(BF16),
            "wbar": wbar.astype(BF16),
            "woT": np.ascontiguousarray(Wo[:, cs].T).astype(BF16),
            "identf": np.eye(128, dtype=np.float32),
            "pbias": np.ascontiguousarray(pb).astype(np.float32),
            "bo_r": np.ascontiguousarray(
                bo_core.reshape(N_DT, 128).T).astype(np.float32),
        })
    return in_maps


def _gather(results, b, n, d):
    acc = np.zeros((d, b * n), dtype=np.float32)
    for r in results:
        acc += r["outT"].astype(np.float32)
    return np.ascontiguousarray(
        acc.reshape(d, b, n).transpose(1, 2, 0)).astype(np.float32)


def kernel(**inputs):
    from concourse.bass_utils import run_bass_kernel_spmd
    x = inputs["x"]
    b, n, d = x.shape
    nc = _get_program(n)
    in_maps = _prep_in_maps(**inputs)
    res = run_bass_kernel_spmd(nc, in_maps, list(range(N_CORES)))
    return _gather(res.results, b, n, d)


def run_profiled(inputs, trace=True):
    from concourse.bass_utils import run_bass_kernel_spmd
    x = inputs["x"]
    b, n, d = x.shape
    nc = _get_program(n)
    in_maps = _prep_in_maps(**inputs)
    res = run_bass_kernel_spmd(nc, in_maps, list(range(N_CORES)), trace=trace)
    return _gather(res.results, b, n, d), res

